# revision 11
# baseline (speedup 1.0000x reference)
"""CenterLoss kernel for 8 TRN2 NeuronCores (v4: sorted-csq tree reduce).

Computes mean over all points of min distance to any center:
    points:  [B=8, N=4096, D=256] f32
    centers: [B=8, K=1024, D=256] f32
    out = mean_{b,n} min_k ||points[b,n] - centers[b,k]||_2

Sharding: data-parallel over B (one batch element per core); host sums the
8 partial sums and divides by B*N.

Per-core algorithm (all fp8e4m3, psq/csq host-precomputed from the
quantized values; HW-calibrated op costs in ns):
    Centers are SORTED by ||c||^2 on host and laid out so that the
    pairwise-max tree's stride-128 "blocks" {j, j+128, ..., j+896} hold 8
    consecutive ranks -> nearly-equal csq within a block. The tree then
    max-reduces RAW cross products (TT-max runs at 2 elem/cycle; a fused
    subtract would force 1x), and a per-block midpoint csq/2 is subtracted
    only at the 128-wide level (block csq spread ~1 -> rel err ~1e-3).

    Per pair of 128-point chunks: 4 DR matmuls (256-deep contraction) into
    a [128, 2, 1024] PSUM tile (4 banks); evacuated to bf16 by ACT (copy,
    ~1.9us/pair) or DVE (tensor_copy) to balance engines. Per group of 8
    chunks: one fused DVE tree [128,8,*]: L1-L3 TT-max (2x), TT-sub cbar,
    L4 TT-max, one 3D tensor_reduce -> mx[:, g*8:g*8+8].
    Epilogue: dist = sqrt(relu(psq - 2*mx)); partial = sum_n dist.
"""

from contextlib import ExitStack

import ml_dtypes
import numpy as np

import concourse.bass as bass
import concourse.mybir as mybir
import concourse.tile as tile
from concourse import bacc
from concourse.bass import ds
from concourse.bass_utils import run_bass_kernel_spmd

B, N, K, D = 8, 4096, 1024, 256
P = 128
NCORES = 8
MCH = N // P     # 32 row-chunks of 128 points
NPAIR = MCH // 2  # 16 chunk-pairs
GRP = 8          # chunks per tree group
NGRP = MCH // GRP
WG = 8           # weight DMA groups
MPG = MCH // WG

F32 = mybir.dt.float32
BF16 = mybir.dt.bfloat16
FP8 = mybir.dt.float8e4
AF = mybir.ActivationFunctionType
ALU = mybir.AluOpType
DR = mybir.MatmulPerfMode.DoubleRow

# pairs whose PSUM is evacuated by DVE tensor_copy instead of ACT, to
# balance ACT (~1.97us/pair) against DVE tree work (~6us/group); keep them
# mid-group and early so they don't collide with tree work on DVE
DVE_EVAC_PAIRS = frozenset({1, 2})

# group sizes (chunks) for the fused tree; smaller final groups shrink the
# post-pipeline tail (tree of the last group runs after the last evac)
GROUP_SIZES = [8, 8, 8, 4, 2, 2]


def _build_kernel(ctx: ExitStack, tc: tile.TileContext, out, wall, cpack_d, cbar8_d, psqT_d):
    nc = tc.nc

    const_pool = ctx.enter_context(tc.tile_pool(name="const", bufs=1))
    psum_main = ctx.enter_context(tc.tile_pool(name="psum_main", bufs=2, space="PSUM"))
    evp = ctx.enter_context(tc.tile_pool(name="evp", bufs=2))
    trp = ctx.enter_context(tc.tile_pool(name="trp", bufs=2))

    # --- bulk input loads -------------------------------------------------
    cpack = const_pool.tile([P, 2, K], FP8, name="cpack", tag="cpack")
    nc.sync.dma_start(cpack[:], cpack_d[:])

    wt = []
    for g in range(WG):
        w = const_pool.tile([P, MPG, 2, P], FP8, name=f"wt{g}", tag=f"wt{g}")
        eng = nc.sync if g % 2 == 0 else nc.gpsimd
        eng.dma_start(w[:], wall[:, ds(g * MPG, MPG), :, :])
        wt.append(w)

    cbar8 = const_pool.tile([P, GRP, P], BF16, name="cbar8", tag="cbar8")
    nc.sync.dma_start(cbar8[:], cbar8_d[:])
    psq = const_pool.tile([P, MCH], F32, name="psq", tag="psq")
    nc.gpsimd.dma_start(psq[:], psqT_d[:])

    onescol = const_pool.tile([P, 1], F32, name="onescol", tag="onescol")
    nc.vector.memset(onescol[:], 1.0)

    mx = const_pool.tile([P, MCH], F32, name="mx", tag="mx")

    # --- PE warm-up: dummy matmuls during the input-DMA window keep the PE
    # clock up so the first real pairs run at full speed
    wz = const_pool.tile([P, 2, P], FP8, name="wz", tag="wz")
    nc.vector.memset(wz[:], 0.0)
    warm = psum_main.tile([P, 2, K], F32, name="warm", tag="pp")
    for i in range(10):
        nc.tensor.matmul(
            warm[:, i % 2, ds(0, P)], wz[:], wz[:],
            start=True, stop=True, perf_mode=DR,
        )

    # --- main loop: 16 chunk-pairs, fused tree per group ------------------
    group_of = []
    group_base = []
    base = 0
    for gi, gs in enumerate(GROUP_SIZES):
        for _ in range(gs):
            group_of.append(gi)
            group_base.append(base)
        base += gs

    ev8 = None
    for t in range(NPAIR):
        m0 = 2 * t
        gi = group_of[m0]
        gs = GROUP_SIZES[gi]
        gb = group_base[m0]
        slot = m0 - gb
        if slot == 0:
            ev8 = evp.tile([P, gs, K], BF16, name="ev8", tag="ev8")

        pp = psum_main.tile([P, 2, K], F32, name="pp", tag="pp")
        for c in range(2):
            m = 2 * t + c
            w = wt[m // MPG][:, m % MPG, :, :]
            for kh in range(K // 512):
                nc.tensor.matmul(
                    pp[:, c, ds(kh * 512, 512)], w, cpack[:, :, ds(kh * 512, 512)],
                    start=True, stop=True, perf_mode=DR,
                )

        evslice = ev8[:, ds(slot, 2), :]
        if t in DVE_EVAC_PAIRS:
            nc.vector.tensor_copy(evslice, pp[:])
        else:
            nc.scalar.copy(evslice, pp[:])

        if slot == gs - 2:
            # fused tree over the whole group
            t1 = trp.tile([P, gs, 512], BF16, name="t1", tag="t1")
            nc.vector.tensor_max(t1[:], ev8[:, :, ds(0, 512)], ev8[:, :, ds(512, 512)])
            t2 = trp.tile([P, gs, 256], BF16, name="t2", tag="t2")
            nc.vector.tensor_max(t2[:], t1[:, :, ds(0, 256)], t1[:, :, ds(256, 256)])
            t3 = trp.tile([P, gs, P], BF16, name="t3", tag="t3")
            nc.vector.tensor_max(t3[:], t2[:, :, ds(0, P)], t2[:, :, ds(P, P)])
            s3 = trp.tile([P, gs, P], BF16, name="s3", tag="s3")
            nc.vector.tensor_sub(s3[:], t3[:], cbar8[:, ds(0, gs), :])
            t4 = trp.tile([P, gs, 64], BF16, name="t4", tag="t4")
            nc.vector.tensor_max(t4[:], s3[:, :, ds(0, 64)], s3[:, :, ds(64, 64)])
            nc.vector.tensor_reduce(
                mx[:, ds(gb, gs)], t4[:], mybir.AxisListType.X, ALU.max
            )

    # --- epilogue: dist = sqrt(relu(psq - 2*mx)); partial = sum dist ------
    d2b = const_pool.tile([P, MCH], F32, name="d2b", tag="d2b")
    nc.vector.scalar_tensor_tensor(d2b[:], mx[:], -2.0, psq[:], ALU.mult, ALU.add)
    d2r = const_pool.tile([P, MCH], F32, name="d2r", tag="d2r")
    nc.vector.tensor_scalar_max(d2r[:], d2b[:], 0.0)
    dist = const_pool.tile([P, MCH], F32, name="dist", tag="dist")
    nc.scalar.activation(dist[:], d2r[:], AF.Sqrt)
    rowsum = const_pool.tile([P, 1], F32, name="rowsum", tag="rowsum")
    nc.vector.tensor_reduce(rowsum[:], dist[:], mybir.AxisListType.X, ALU.add)
    fin = psum_main.tile([1, 1, 1], F32, name="fin", tag="pp", padded_shape=[P, 2, K])
    nc.tensor.matmul(fin[:], rowsum[:], onescol[:], start=True, stop=True)
    out_sb = const_pool.tile([1, 1], F32, name="out_sb", tag="out_sb")
    nc.scalar.copy(out_sb[:], fin[:])
    nc.gpsimd.dma_start(out[:], out_sb[:])


def build():
    nc = bacc.Bacc(
        "TRN2",
        target_bir_lowering=False,
        debug=False,
        enable_asserts=False,
        num_devices=NCORES,
    )
    wall = nc.dram_tensor("wall", [P, MCH, 2, P], FP8, kind="ExternalInput").ap()
    cpack_d = nc.dram_tensor("cpack", [P, 2, K], FP8, kind="ExternalInput").ap()
    cbar8_d = nc.dram_tensor("cbar8", [P, GRP, P], BF16, kind="ExternalInput").ap()
    psqT_d = nc.dram_tensor("psqT", [P, MCH], F32, kind="ExternalInput").ap()
    out = nc.dram_tensor("out", [1, 1], F32, kind="ExternalOutput").ap()
    with tile.TileContext(nc) as tc, ExitStack() as ctx:
        _build_kernel(ctx, tc, out, wall, cpack_d, cbar8_d, psqT_d)
    nc.compile()
    return nc


_NC = None


def _make_in_maps(points: np.ndarray, centers: np.ndarray):
    # column c of cpack holds sorted-rank r = (c % 128)*8 + c//128, so the
    # max-tree block {j, j+128, ..., j+896} covers ranks 8j..8j+7
    cols = np.arange(K)
    rank_of_col = (cols % P) * GRP + cols // P
    in_maps = []
    for b in range(B):
        p8 = points[b].astype(ml_dtypes.float8_e4m3)    # [N, D]
        c8 = centers[b].astype(ml_dtypes.float8_e4m3)   # [K, D]
        pf = p8.astype(np.float32)
        cf = c8.astype(np.float32)
        psq = np.einsum("nd,nd->n", pf, pf)             # [N]
        csqh = 0.5 * np.einsum("kd,kd->k", cf, cf)      # [K]
        order = np.argsort(csqh)
        c8i = c8[order][rank_of_col]                    # centers in column order
        blk = csqh[order].reshape(P, GRP)               # block j = ranks 8j..8j+7
        cbar = 0.5 * (blk.min(1) + blk.max(1))          # [128] midpoint
        cbar8 = np.broadcast_to(
            cbar.astype(ml_dtypes.bfloat16)[None, None, :], (P, GRP, P)
        )
        # wall[p, m, s, n] = p8[m*128+n, s*128+p]
        wall = np.ascontiguousarray(
            p8.reshape(MCH, P, 2, P).transpose(3, 0, 2, 1)
        )
        # cpack[p, s, k] = c8i[k, s*128+p]
        cpack = np.ascontiguousarray(c8i.reshape(K, 2, P).transpose(2, 1, 0))
        in_maps.append(
            {
                "wall": wall,
                "cpack": cpack,
                "cbar8": np.ascontiguousarray(cbar8),
                "psqT": np.ascontiguousarray(psq.reshape(MCH, P).T),
            }
        )
    return in_maps


def kernel(points, centers, **_run_kwargs):
    global _NC
    points = np.asarray(points, dtype=np.float32)
    centers = np.asarray(centers, dtype=np.float32)
    assert points.shape == (B, N, D) and centers.shape == (B, K, D)
    if _NC is None:
        _NC = build()
    res = run_bass_kernel_spmd(
        _NC, _make_in_maps(points, centers), list(range(NCORES)), **_run_kwargs
    )
    total = sum(float(r["out"][0, 0]) for r in res.results)
    return np.array(total / (B * N), dtype=np.float32)


if __name__ == "__main__":
    pts = np.random.RandomState(0).randn(B, N, D).astype(np.float32)
    ctr = np.random.RandomState(1).randn(B, K, D).astype(np.float32)
    print(kernel(pts, ctr))


# revision 12
# speedup vs baseline: 1.0306x; 1.0306x over previous
"""CenterLoss kernel for 8 TRN2 NeuronCores (v4: sorted-csq tree reduce).

Computes mean over all points of min distance to any center:
    points:  [B=8, N=4096, D=256] f32
    centers: [B=8, K=1024, D=256] f32
    out = mean_{b,n} min_k ||points[b,n] - centers[b,k]||_2

Sharding: data-parallel over B (one batch element per core); host sums the
8 partial sums and divides by B*N.

Per-core algorithm (all fp8e4m3, psq/csq host-precomputed from the
quantized values; HW-calibrated op costs in ns):
    Centers are SORTED by ||c||^2 on host and laid out so that the
    pairwise-max tree's stride-128 "blocks" {j, j+128, ..., j+896} hold 8
    consecutive ranks -> nearly-equal csq within a block. The tree then
    max-reduces RAW cross products (TT-max runs at 2 elem/cycle; a fused
    subtract would force 1x), and a per-block midpoint csq/2 is subtracted
    only at the 128-wide level (block csq spread ~1 -> rel err ~1e-3).

    Per pair of 128-point chunks: 4 DR matmuls (256-deep contraction) into
    a [128, 2, 1024] PSUM tile (4 banks); evacuated to bf16 by ACT (copy,
    ~1.9us/pair) or DVE (tensor_copy) to balance engines. Per group of 8
    chunks: one fused DVE tree [128,8,*]: L1-L3 TT-max (2x), TT-sub cbar,
    L4 TT-max, one 3D tensor_reduce -> mx[:, g*8:g*8+8].
    Epilogue: dist = sqrt(relu(psq - 2*mx)); partial = sum_n dist.
"""

from contextlib import ExitStack

import ml_dtypes
import numpy as np

import concourse.bass as bass
import concourse.mybir as mybir
import concourse.tile as tile
from concourse import bacc
from concourse.bass import ds
from concourse.bass_utils import run_bass_kernel_spmd

B, N, K, D = 8, 4096, 1024, 256
P = 128
NCORES = 8
MCH = N // P     # 32 row-chunks of 128 points
NPAIR = MCH // 2  # 16 chunk-pairs
GRP = 8          # chunks per tree group
NGRP = MCH // GRP
WG = 4           # weight DMA groups
MPG = MCH // WG

F32 = mybir.dt.float32
BF16 = mybir.dt.bfloat16
FP8 = mybir.dt.float8e4
AF = mybir.ActivationFunctionType
ALU = mybir.AluOpType
DR = mybir.MatmulPerfMode.DoubleRow

# pairs whose PSUM is evacuated by DVE tensor_copy instead of ACT, to
# balance ACT (~1.97us/pair) against DVE tree work (~6us/group); keep them
# mid-group and early so they don't collide with tree work on DVE
DVE_EVAC_PAIRS = frozenset()

# group sizes (chunks) for the fused tree; smaller final groups shrink the
# post-pipeline tail (tree of the last group runs after the last evac)
GROUP_SIZES = [8, 8, 8, 4, 2, 2]


def _build_kernel(ctx: ExitStack, tc: tile.TileContext, out, wall, cpack_d, cbar8_d, psqT_d):
    nc = tc.nc

    const_pool = ctx.enter_context(tc.tile_pool(name="const", bufs=1))
    psum_main = ctx.enter_context(tc.tile_pool(name="psum_main", bufs=2, space="PSUM"))
    evp = ctx.enter_context(tc.tile_pool(name="evp", bufs=2))
    trp = ctx.enter_context(tc.tile_pool(name="trp", bufs=2))

    # --- bulk input loads (first weight piece first: it gates pair 0) -----
    wt = [
        const_pool.tile([P, MPG, 2, P], FP8, name=f"wt{g}", tag=f"wt{g}")
        for g in range(WG)
    ]
    nc.sync.dma_start(wt[0][:], wall[:, ds(0, MPG), :, :])
    cpack = const_pool.tile([P, 2, K], FP8, name="cpack", tag="cpack")
    nc.gpsimd.dma_start(cpack[:], cpack_d[:])
    for g in range(1, WG):
        eng = nc.sync if g % 2 == 1 else nc.gpsimd
        eng.dma_start(wt[g][:], wall[:, ds(g * MPG, MPG), :, :])

    cbar8 = const_pool.tile([P, GRP, P], BF16, name="cbar8", tag="cbar8")
    nc.sync.dma_start(cbar8[:], cbar8_d[:])
    psq = const_pool.tile([P, MCH], F32, name="psq", tag="psq")
    nc.gpsimd.dma_start(psq[:], psqT_d[:])

    onescol = const_pool.tile([P, 1], F32, name="onescol", tag="onescol")
    nc.vector.memset(onescol[:], 1.0)

    mx = const_pool.tile([P, MCH], F32, name="mx", tag="mx")
    d2b = const_pool.tile([P, MCH], F32, name="d2b", tag="d2b")
    d2r = const_pool.tile([P, MCH], F32, name="d2r", tag="d2r")
    dist = const_pool.tile([P, MCH], F32, name="dist", tag="dist")

    # --- PE warm-up: dummy matmuls during the input-DMA window keep the PE
    # clock up so the first real pairs run at full speed
    wz = const_pool.tile([P, 2, P], FP8, name="wz", tag="wz")
    nc.vector.memset(wz[:], 0.0)
    warm = psum_main.tile([P, 2, K], F32, name="warm", tag="pp")
    for i in range(10):
        nc.tensor.matmul(
            warm[:, i % 2, ds(0, P)], wz[:], wz[:],
            start=True, stop=True, perf_mode=DR,
        )

    # --- main loop: 16 chunk-pairs, fused tree per group ------------------
    group_of = []
    group_base = []
    base = 0
    for gi, gs in enumerate(GROUP_SIZES):
        for _ in range(gs):
            group_of.append(gi)
            group_base.append(base)
        base += gs

    ev8 = None
    for t in range(NPAIR):
        m0 = 2 * t
        gi = group_of[m0]
        gs = GROUP_SIZES[gi]
        gb = group_base[m0]
        slot = m0 - gb
        if slot == 0:
            ev8 = evp.tile([P, gs, K], BF16, name="ev8", tag="ev8")

        pp = psum_main.tile([P, 2, K], F32, name="pp", tag="pp")
        for c in range(2):
            m = 2 * t + c
            w = wt[m // MPG][:, m % MPG, :, :]
            for kh in range(K // 512):
                nc.tensor.matmul(
                    pp[:, c, ds(kh * 512, 512)], w, cpack[:, :, ds(kh * 512, 512)],
                    start=True, stop=True, perf_mode=DR,
                )

        evslice = ev8[:, ds(slot, 2), :]
        if t in DVE_EVAC_PAIRS:
            nc.vector.tensor_copy(evslice, pp[:])
        else:
            nc.scalar.copy(evslice, pp[:])

        if slot == gs - 2:
            # fused tree over the whole group
            t1 = trp.tile([P, gs, 512], BF16, name="t1", tag="t1")
            nc.vector.tensor_max(t1[:], ev8[:, :, ds(0, 512)], ev8[:, :, ds(512, 512)])
            t2 = trp.tile([P, gs, 256], BF16, name="t2", tag="t2")
            nc.vector.tensor_max(t2[:], t1[:, :, ds(0, 256)], t1[:, :, ds(256, 256)])
            t3 = trp.tile([P, gs, P], BF16, name="t3", tag="t3")
            nc.vector.tensor_max(t3[:], t2[:, :, ds(0, P)], t2[:, :, ds(P, P)])
            s3 = trp.tile([P, gs, P], BF16, name="s3", tag="s3")
            nc.vector.tensor_sub(s3[:], t3[:], cbar8[:, ds(0, gs), :])
            t4 = trp.tile([P, gs, 64], BF16, name="t4", tag="t4")
            nc.vector.tensor_max(t4[:], s3[:, :, ds(0, 64)], s3[:, :, ds(64, 64)])
            nc.vector.tensor_reduce(
                mx[:, ds(gb, gs)], t4[:], mybir.AxisListType.X, ALU.max
            )
            # epilogue slice for this group while the pipeline continues:
            # dist = sqrt(relu(psq - 2*mx))
            sl = ds(gb, gs)
            nc.vector.scalar_tensor_tensor(
                d2b[:, sl], mx[:, sl], -2.0, psq[:, sl], ALU.mult, ALU.add
            )
            nc.vector.tensor_scalar_max(d2r[:, sl], d2b[:, sl], 0.0)
            nc.scalar.activation(dist[:, sl], d2r[:, sl], AF.Sqrt)

    # --- epilogue: partial = sum_n dist ----------------------------------
    rowsum = const_pool.tile([P, 1], F32, name="rowsum", tag="rowsum")
    nc.vector.tensor_reduce(rowsum[:], dist[:], mybir.AxisListType.X, ALU.add)
    fin = psum_main.tile([1, 1, 1], F32, name="fin", tag="pp", padded_shape=[P, 2, K])
    nc.tensor.matmul(fin[:], rowsum[:], onescol[:], start=True, stop=True)
    out_sb = const_pool.tile([1, 1], F32, name="out_sb", tag="out_sb")
    nc.scalar.copy(out_sb[:], fin[:])
    nc.gpsimd.dma_start(out[:], out_sb[:])


def build():
    nc = bacc.Bacc(
        "TRN2",
        target_bir_lowering=False,
        debug=False,
        enable_asserts=False,
        num_devices=NCORES,
    )
    wall = nc.dram_tensor("wall", [P, MCH, 2, P], FP8, kind="ExternalInput").ap()
    cpack_d = nc.dram_tensor("cpack", [P, 2, K], FP8, kind="ExternalInput").ap()
    cbar8_d = nc.dram_tensor("cbar8", [P, GRP, P], BF16, kind="ExternalInput").ap()
    psqT_d = nc.dram_tensor("psqT", [P, MCH], F32, kind="ExternalInput").ap()
    out = nc.dram_tensor("out", [1, 1], F32, kind="ExternalOutput").ap()
    with tile.TileContext(nc) as tc, ExitStack() as ctx:
        _build_kernel(ctx, tc, out, wall, cpack_d, cbar8_d, psqT_d)
    nc.compile()
    return nc


_NC = None


def _make_in_maps(points: np.ndarray, centers: np.ndarray):
    # column c of cpack holds sorted-rank r = (c % 128)*8 + c//128, so the
    # max-tree block {j, j+128, ..., j+896} covers ranks 8j..8j+7
    cols = np.arange(K)
    rank_of_col = (cols % P) * GRP + cols // P
    in_maps = []
    for b in range(B):
        p8 = points[b].astype(ml_dtypes.float8_e4m3)    # [N, D]
        c8 = centers[b].astype(ml_dtypes.float8_e4m3)   # [K, D]
        pf = p8.astype(np.float32)
        cf = c8.astype(np.float32)
        psq = np.einsum("nd,nd->n", pf, pf)             # [N]
        csqh = 0.5 * np.einsum("kd,kd->k", cf, cf)      # [K]
        order = np.argsort(csqh)
        c8i = c8[order][rank_of_col]                    # centers in column order
        blk = csqh[order].reshape(P, GRP)               # block j = ranks 8j..8j+7
        cbar = 0.5 * (blk.min(1) + blk.max(1))          # [128] midpoint
        cbar8 = np.broadcast_to(
            cbar.astype(ml_dtypes.bfloat16)[None, None, :], (P, GRP, P)
        )
        # wall[p, m, s, n] = p8[m*128+n, s*128+p]
        wall = np.ascontiguousarray(
            p8.reshape(MCH, P, 2, P).transpose(3, 0, 2, 1)
        )
        # cpack[p, s, k] = c8i[k, s*128+p]
        cpack = np.ascontiguousarray(c8i.reshape(K, 2, P).transpose(2, 1, 0))
        in_maps.append(
            {
                "wall": wall,
                "cpack": cpack,
                "cbar8": np.ascontiguousarray(cbar8),
                "psqT": np.ascontiguousarray(psq.reshape(MCH, P).T),
            }
        )
    return in_maps


def kernel(points, centers, **_run_kwargs):
    global _NC
    points = np.asarray(points, dtype=np.float32)
    centers = np.asarray(centers, dtype=np.float32)
    assert points.shape == (B, N, D) and centers.shape == (B, K, D)
    if _NC is None:
        _NC = build()
    res = run_bass_kernel_spmd(
        _NC, _make_in_maps(points, centers), list(range(NCORES)), **_run_kwargs
    )
    total = sum(float(r["out"][0, 0]) for r in res.results)
    return np.array(total / (B * N), dtype=np.float32)


if __name__ == "__main__":
    pts = np.random.RandomState(0).randn(B, N, D).astype(np.float32)
    ctr = np.random.RandomState(1).randn(B, K, D).astype(np.float32)
    print(kernel(pts, ctr))


# revision 14
# speedup vs baseline: 1.0565x; 1.0252x over previous
"""CenterLoss kernel for 8 TRN2 NeuronCores (v4: sorted-csq tree reduce).

Computes mean over all points of min distance to any center:
    points:  [B=8, N=4096, D=256] f32
    centers: [B=8, K=1024, D=256] f32
    out = mean_{b,n} min_k ||points[b,n] - centers[b,k]||_2

Sharding: data-parallel over B (one batch element per core); host sums the
8 partial sums and divides by B*N.

Per-core algorithm (all fp8e4m3, psq/csq host-precomputed from the
quantized values; HW-calibrated op costs in ns):
    Centers are SORTED by ||c||^2 on host and laid out so that the
    pairwise-max tree's stride-128 "blocks" {j, j+128, ..., j+896} hold 8
    consecutive ranks -> nearly-equal csq within a block. The tree then
    max-reduces RAW cross products (TT-max runs at 2 elem/cycle; a fused
    subtract would force 1x), and a per-block midpoint csq/2 is subtracted
    only at the 128-wide level (block csq spread ~1 -> rel err ~1e-3).

    Per pair of 128-point chunks: 4 DR matmuls (256-deep contraction) into
    a [128, 2, 1024] PSUM tile (4 banks); evacuated to bf16 by ACT (copy,
    ~1.9us/pair) or DVE (tensor_copy) to balance engines. Per group of 8
    chunks: one fused DVE tree [128,8,*]: L1-L3 TT-max (2x), TT-sub cbar,
    L4 TT-max, one 3D tensor_reduce -> mx[:, g*8:g*8+8].
    Epilogue: dist = sqrt(relu(psq - 2*mx)); partial = sum_n dist.
"""

from contextlib import ExitStack

import ml_dtypes
import numpy as np

import concourse.bass as bass
import concourse.mybir as mybir
import concourse.tile as tile
from concourse import bacc
from concourse.bass import ds
from concourse.bass_utils import run_bass_kernel_spmd

B, N, K, D = 8, 4096, 1024, 256
P = 128
NCORES = 8
MCH = N // P     # 32 row-chunks of 128 points
NPAIR = MCH // 2  # 16 chunk-pairs
GRP = 8          # chunks per tree group
NGRP = MCH // GRP
WG = 4           # weight DMA groups
MPG = MCH // WG

F32 = mybir.dt.float32
BF16 = mybir.dt.bfloat16
FP8 = mybir.dt.float8e4
AF = mybir.ActivationFunctionType
ALU = mybir.AluOpType
DR = mybir.MatmulPerfMode.DoubleRow

# pairs whose PSUM is evacuated by DVE tensor_copy instead of ACT, to
# balance ACT (~1.97us/pair) against DVE tree work (~6us/group); keep them
# mid-group and early so they don't collide with tree work on DVE
DVE_EVAC_PAIRS = frozenset()

# group sizes (chunks) for the fused tree; smaller final groups shrink the
# post-pipeline tail (tree of the last group runs after the last evac)
GROUP_SIZES = [8, 8, 8, 4, 4]


def _build_kernel(ctx: ExitStack, tc: tile.TileContext, out, wall, cpack_d, cbar8_d, psqT_d):
    nc = tc.nc

    const_pool = ctx.enter_context(tc.tile_pool(name="const", bufs=1))
    psum_main = ctx.enter_context(tc.tile_pool(name="psum_main", bufs=2, space="PSUM"))
    evp = ctx.enter_context(tc.tile_pool(name="evp", bufs=2))
    trp = ctx.enter_context(tc.tile_pool(name="trp", bufs=2))

    # --- bulk input loads. DMA landing latency is per-partition-packet
    # bound (~43ns/packet), so the two tensors that gate pair 0 (cpack +
    # first wall piece) are split into partition halves across 4 queues.
    wt = [
        const_pool.tile([P, MPG, 2, P], FP8, name=f"wt{g}", tag=f"wt{g}")
        for g in range(WG)
    ]
    cpack = const_pool.tile([P, 2, K], FP8, name="cpack", tag="cpack")
    nc.sync.dma_start(cpack[ds(0, 64), :, :], cpack_d[ds(0, 64), :, :])
    nc.gpsimd.dma_start(cpack[ds(64, 64), :, :], cpack_d[ds(64, 64), :, :])
    nc.scalar.dma_start(wt[0][ds(0, 64), :, :, :], wall[ds(0, 64), ds(0, MPG), :, :])
    nc.sync.dma_start(wt[0][ds(64, 64), :, :, :], wall[ds(64, 64), ds(0, MPG), :, :])
    for g in range(1, WG):
        eng = nc.sync if g % 2 == 1 else nc.gpsimd
        eng.dma_start(wt[g][:], wall[:, ds(g * MPG, MPG), :, :])

    cbar8 = const_pool.tile([P, GRP, P], BF16, name="cbar8", tag="cbar8")
    nc.sync.dma_start(cbar8[:], cbar8_d[:])
    psq = const_pool.tile([P, MCH], F32, name="psq", tag="psq")
    nc.gpsimd.dma_start(psq[:], psqT_d[:])

    onescol = const_pool.tile([P, 1], F32, name="onescol", tag="onescol")
    nc.vector.memset(onescol[:], 1.0)

    mx = const_pool.tile([P, MCH], F32, name="mx", tag="mx")
    d2b = const_pool.tile([P, MCH], F32, name="d2b", tag="d2b")
    d2r = const_pool.tile([P, MCH], F32, name="d2r", tag="d2r")
    dist = const_pool.tile([P, MCH], F32, name="dist", tag="dist")

    # --- PE warm-up: dummy matmuls during the input-DMA window keep the PE
    # clock up so the first real pairs run at full speed
    wz = const_pool.tile([P, 2, P], FP8, name="wz", tag="wz")
    nc.vector.memset(wz[:], 0.0)
    warm = psum_main.tile([P, 2, K], F32, name="warm", tag="pp")
    for i in range(16):
        nc.tensor.matmul(
            warm[:, i % 2, ds(0, P)], wz[:], wz[:],
            start=True, stop=True, perf_mode=DR,
        )

    # --- main loop: 16 chunk-pairs, fused tree per group ------------------
    group_of = []
    group_base = []
    base = 0
    for gi, gs in enumerate(GROUP_SIZES):
        for _ in range(gs):
            group_of.append(gi)
            group_base.append(base)
        base += gs

    ev8 = None
    for t in range(NPAIR):
        m0 = 2 * t
        gi = group_of[m0]
        gs = GROUP_SIZES[gi]
        gb = group_base[m0]
        slot = m0 - gb
        if slot == 0:
            ev8 = evp.tile([P, gs, K], BF16, name="ev8", tag="ev8")

        pp = psum_main.tile([P, 2, K], F32, name="pp", tag="pp")
        for c in range(2):
            m = 2 * t + c
            w = wt[m // MPG][:, m % MPG, :, :]
            for kh in range(K // 512):
                nc.tensor.matmul(
                    pp[:, c, ds(kh * 512, 512)], w, cpack[:, :, ds(kh * 512, 512)],
                    start=True, stop=True, perf_mode=DR,
                )

        evslice = ev8[:, ds(slot, 2), :]
        if t in DVE_EVAC_PAIRS:
            nc.vector.tensor_copy(evslice, pp[:])
        else:
            nc.scalar.copy(evslice, pp[:])

        # L1 per pair so the group tree has a short post-evac tail
        if slot == 0:
            t1 = trp.tile([P, gs, 512], BF16, name="t1", tag="t1")
        nc.vector.tensor_max(
            t1[:, ds(slot, 2), :], evslice[:, :, ds(0, 512)], evslice[:, :, ds(512, 512)]
        )

        if slot == gs - 2:
            # fused tree (from L2) over the whole group
            t2 = trp.tile([P, gs, 256], BF16, name="t2", tag="t2")
            nc.vector.tensor_max(t2[:], t1[:, :, ds(0, 256)], t1[:, :, ds(256, 256)])
            t3 = trp.tile([P, gs, P], BF16, name="t3", tag="t3")
            nc.vector.tensor_max(t3[:], t2[:, :, ds(0, P)], t2[:, :, ds(P, P)])
            s3 = trp.tile([P, gs, P], BF16, name="s3", tag="s3")
            nc.vector.tensor_sub(s3[:], t3[:], cbar8[:, ds(0, gs), :])
            t4 = trp.tile([P, gs, 64], BF16, name="t4", tag="t4")
            nc.vector.tensor_max(t4[:], s3[:, :, ds(0, 64)], s3[:, :, ds(64, 64)])
            nc.vector.tensor_reduce(
                mx[:, ds(gb, gs)], t4[:], mybir.AxisListType.X, ALU.max
            )
            # epilogue slice for this group while the pipeline continues:
            # dist = sqrt(relu(psq - 2*mx))
            sl = ds(gb, gs)
            nc.vector.scalar_tensor_tensor(
                d2b[:, sl], mx[:, sl], -2.0, psq[:, sl], ALU.mult, ALU.add
            )
            nc.vector.tensor_scalar_max(d2r[:, sl], d2b[:, sl], 0.0)
            nc.scalar.activation(dist[:, sl], d2r[:, sl], AF.Sqrt)

    # --- epilogue: partial = sum_n dist ----------------------------------
    rowsum = const_pool.tile([P, 1], F32, name="rowsum", tag="rowsum")
    nc.vector.tensor_reduce(rowsum[:], dist[:], mybir.AxisListType.X, ALU.add)
    fin = psum_main.tile([1, 1, 1], F32, name="fin", tag="pp", padded_shape=[P, 2, K])
    nc.tensor.matmul(fin[:], rowsum[:], onescol[:], start=True, stop=True)
    out_sb = const_pool.tile([1, 1], F32, name="out_sb", tag="out_sb")
    nc.scalar.copy(out_sb[:], fin[:])
    nc.gpsimd.dma_start(out[:], out_sb[:])


def build():
    nc = bacc.Bacc(
        "TRN2",
        target_bir_lowering=False,
        debug=False,
        enable_asserts=False,
        num_devices=NCORES,
    )
    wall = nc.dram_tensor("wall", [P, MCH, 2, P], FP8, kind="ExternalInput").ap()
    cpack_d = nc.dram_tensor("cpack", [P, 2, K], FP8, kind="ExternalInput").ap()
    cbar8_d = nc.dram_tensor("cbar8", [P, GRP, P], BF16, kind="ExternalInput").ap()
    psqT_d = nc.dram_tensor("psqT", [P, MCH], F32, kind="ExternalInput").ap()
    out = nc.dram_tensor("out", [1, 1], F32, kind="ExternalOutput").ap()
    with tile.TileContext(nc) as tc, ExitStack() as ctx:
        _build_kernel(ctx, tc, out, wall, cpack_d, cbar8_d, psqT_d)
    nc.compile()
    return nc


_NC = None


def _make_in_maps(points: np.ndarray, centers: np.ndarray):
    # column c of cpack holds sorted-rank r = (c % 128)*8 + c//128, so the
    # max-tree block {j, j+128, ..., j+896} covers ranks 8j..8j+7
    cols = np.arange(K)
    rank_of_col = (cols % P) * GRP + cols // P
    in_maps = []
    for b in range(B):
        p8 = points[b].astype(ml_dtypes.float8_e4m3)    # [N, D]
        c8 = centers[b].astype(ml_dtypes.float8_e4m3)   # [K, D]
        pf = p8.astype(np.float32)
        cf = c8.astype(np.float32)
        psq = np.einsum("nd,nd->n", pf, pf)             # [N]
        csqh = 0.5 * np.einsum("kd,kd->k", cf, cf)      # [K]
        order = np.argsort(csqh)
        c8i = c8[order][rank_of_col]                    # centers in column order
        blk = csqh[order].reshape(P, GRP)               # block j = ranks 8j..8j+7
        cbar = 0.5 * (blk.min(1) + blk.max(1))          # [128] midpoint
        cbar8 = np.broadcast_to(
            cbar.astype(ml_dtypes.bfloat16)[None, None, :], (P, GRP, P)
        )
        # wall[p, m, s, n] = p8[m*128+n, s*128+p]
        wall = np.ascontiguousarray(
            p8.reshape(MCH, P, 2, P).transpose(3, 0, 2, 1)
        )
        # cpack[p, s, k] = c8i[k, s*128+p]
        cpack = np.ascontiguousarray(c8i.reshape(K, 2, P).transpose(2, 1, 0))
        in_maps.append(
            {
                "wall": wall,
                "cpack": cpack,
                "cbar8": np.ascontiguousarray(cbar8),
                "psqT": np.ascontiguousarray(psq.reshape(MCH, P).T),
            }
        )
    return in_maps


def kernel(points, centers, **_run_kwargs):
    global _NC
    points = np.asarray(points, dtype=np.float32)
    centers = np.asarray(centers, dtype=np.float32)
    assert points.shape == (B, N, D) and centers.shape == (B, K, D)
    if _NC is None:
        _NC = build()
    res = run_bass_kernel_spmd(
        _NC, _make_in_maps(points, centers), list(range(NCORES)), **_run_kwargs
    )
    total = sum(float(r["out"][0, 0]) for r in res.results)
    return np.array(total / (B * N), dtype=np.float32)


if __name__ == "__main__":
    pts = np.random.RandomState(0).randn(B, N, D).astype(np.float32)
    ctr = np.random.RandomState(1).randn(B, K, D).astype(np.float32)
    print(kernel(pts, ctr))


# revision 15
# speedup vs baseline: 1.0938x; 1.0353x over previous
"""CenterLoss kernel for 8 TRN2 NeuronCores (v4: sorted-csq tree reduce).

Computes mean over all points of min distance to any center:
    points:  [B=8, N=4096, D=256] f32
    centers: [B=8, K=1024, D=256] f32
    out = mean_{b,n} min_k ||points[b,n] - centers[b,k]||_2

Sharding: data-parallel over B (one batch element per core); host sums the
8 partial sums and divides by B*N.

Per-core algorithm (all fp8e4m3, psq/csq host-precomputed from the
quantized values; HW-calibrated op costs in ns):
    Centers are SORTED by ||c||^2 on host and laid out so that the
    pairwise-max tree's stride-128 "blocks" {j, j+128, ..., j+896} hold 8
    consecutive ranks -> nearly-equal csq within a block. The tree then
    max-reduces RAW cross products (TT-max runs at 2 elem/cycle; a fused
    subtract would force 1x), and a per-block midpoint csq/2 is subtracted
    only at the 128-wide level (block csq spread ~1 -> rel err ~1e-3).

    Per pair of 128-point chunks: 4 DR matmuls (256-deep contraction) into
    a [128, 2, 1024] PSUM tile (4 banks); evacuated to bf16 by ACT (copy,
    ~1.9us/pair) or DVE (tensor_copy) to balance engines. Per group of 8
    chunks: one fused DVE tree [128,8,*]: L1-L3 TT-max (2x), TT-sub cbar,
    L4 TT-max, one 3D tensor_reduce -> mx[:, g*8:g*8+8].
    Epilogue: dist = sqrt(relu(psq - 2*mx)); partial = sum_n dist.
"""

from contextlib import ExitStack

import ml_dtypes
import numpy as np

import concourse.bass as bass
import concourse.mybir as mybir
import concourse.tile as tile
from concourse import bacc
from concourse.bass import ds
from concourse.bass_utils import run_bass_kernel_spmd

B, N, K, D = 8, 4096, 1024, 256
P = 128
NCORES = 8
MCH = N // P     # 32 row-chunks of 128 points
NPAIR = MCH // 2  # 16 chunk-pairs
GRP = 8          # chunks per tree group
NGRP = MCH // GRP
WG = 4           # weight DMA groups
MPG = MCH // WG

F32 = mybir.dt.float32
BF16 = mybir.dt.bfloat16
FP8 = mybir.dt.float8e4
AF = mybir.ActivationFunctionType
ALU = mybir.AluOpType
DR = mybir.MatmulPerfMode.DoubleRow

# pairs whose PSUM is evacuated by DVE tensor_copy instead of ACT, to
# balance ACT (~1.97us/pair) against DVE tree work (~6us/group); keep them
# mid-group and early so they don't collide with tree work on DVE
DVE_EVAC_PAIRS = frozenset()

# group sizes (chunks) for the fused tree; smaller final groups shrink the
# post-pipeline tail (tree of the last group runs after the last evac)
GROUP_SIZES = [8, 8, 8, 4, 4]


def _build_kernel(ctx: ExitStack, tc: tile.TileContext, out, wall, cpack_d, cbar8_d, psqT_d):
    nc = tc.nc

    const_pool = ctx.enter_context(tc.tile_pool(name="const", bufs=1))
    psum_main = ctx.enter_context(tc.tile_pool(name="psum_main", bufs=2, space="PSUM"))
    evp = ctx.enter_context(tc.tile_pool(name="evp", bufs=2))
    trp = ctx.enter_context(tc.tile_pool(name="trp", bufs=2))

    # --- bulk input loads. DMA landing latency is per-partition-packet
    # bound (~43ns/packet), so the two tensors that gate pair 0 (cpack +
    # first wall piece) are split into partition halves across 4 queues.
    wt = [
        const_pool.tile([P, MPG, 2, P], FP8, name=f"wt{g}", tag=f"wt{g}")
        for g in range(WG)
    ]
    cpack = const_pool.tile([P, 2, K], FP8, name="cpack", tag="cpack")
    nc.sync.dma_start(cpack[ds(0, 64), :, :], cpack_d[ds(0, 64), :, :])
    nc.gpsimd.dma_start(cpack[ds(64, 64), :, :], cpack_d[ds(64, 64), :, :])
    nc.scalar.dma_start(wt[0][ds(0, 64), :, :, :], wall[ds(0, 64), ds(0, MPG), :, :])
    nc.sync.dma_start(wt[0][ds(64, 64), :, :, :], wall[ds(64, 64), ds(0, MPG), :, :])
    for g in range(1, WG):
        eng = nc.sync if g % 2 == 1 else nc.gpsimd
        eng.dma_start(wt[g][:], wall[:, ds(g * MPG, MPG), :, :])

    cbar8 = const_pool.tile([P, GRP, P], BF16, name="cbar8", tag="cbar8")
    nc.sync.dma_start(cbar8[:], cbar8_d[:])
    psq = const_pool.tile([P, MCH], F32, name="psq", tag="psq")
    nc.gpsimd.dma_start(psq[:], psqT_d[:])

    onescol = const_pool.tile([P, 1], F32, name="onescol", tag="onescol")
    nc.vector.memset(onescol[:], 1.0)

    mx = const_pool.tile([P, MCH], F32, name="mx", tag="mx")
    d2b = const_pool.tile([P, MCH], F32, name="d2b", tag="d2b")
    d2r = const_pool.tile([P, MCH], F32, name="d2r", tag="d2r")
    dist = const_pool.tile([P, MCH], F32, name="dist", tag="dist")

    # --- PE warm-up: dummy matmuls during the input-DMA window keep the PE
    # clock up so the first real pairs run at full speed
    wz = const_pool.tile([P, 2, P], FP8, name="wz", tag="wz")
    nc.vector.memset(wz[:], 0.0)
    warm = psum_main.tile([P, 2, K], F32, name="warm", tag="pp")
    for i in range(26):
        nc.tensor.matmul(
            warm[:, i % 2, ds(0, P)], wz[:], wz[:],
            start=True, stop=True, perf_mode=DR,
        )

    # --- main loop: 16 chunk-pairs, fused tree per group ------------------
    group_of = []
    group_base = []
    base = 0
    for gi, gs in enumerate(GROUP_SIZES):
        for _ in range(gs):
            group_of.append(gi)
            group_base.append(base)
        base += gs

    ev8 = None
    for t in range(NPAIR):
        m0 = 2 * t
        gi = group_of[m0]
        gs = GROUP_SIZES[gi]
        gb = group_base[m0]
        slot = m0 - gb
        if slot == 0:
            ev8 = evp.tile([P, gs, K], BF16, name="ev8", tag="ev8")

        pp = psum_main.tile([P, 2, K], F32, name="pp", tag="pp")
        for c in range(2):
            m = 2 * t + c
            w = wt[m // MPG][:, m % MPG, :, :]
            for kh in range(K // 512):
                nc.tensor.matmul(
                    pp[:, c, ds(kh * 512, 512)], w, cpack[:, :, ds(kh * 512, 512)],
                    start=True, stop=True, perf_mode=DR,
                )

        evslice = ev8[:, ds(slot, 2), :]
        if t in DVE_EVAC_PAIRS:
            nc.vector.tensor_copy(evslice, pp[:])
        else:
            nc.scalar.copy(evslice, pp[:])

        # L1 per pair so the group tree has a short post-evac tail
        if slot == 0:
            t1 = trp.tile([P, gs, 512], BF16, name="t1", tag="t1")
        nc.vector.tensor_max(
            t1[:, ds(slot, 2), :], evslice[:, :, ds(0, 512)], evslice[:, :, ds(512, 512)]
        )

        if slot == gs - 2:
            # fused tree (from L2) over the whole group
            t2 = trp.tile([P, gs, 256], BF16, name="t2", tag="t2")
            nc.vector.tensor_max(t2[:], t1[:, :, ds(0, 256)], t1[:, :, ds(256, 256)])
            t3 = trp.tile([P, gs, P], BF16, name="t3", tag="t3")
            nc.vector.tensor_max(t3[:], t2[:, :, ds(0, P)], t2[:, :, ds(P, P)])
            s3 = trp.tile([P, gs, P], BF16, name="s3", tag="s3")
            nc.vector.tensor_sub(s3[:], t3[:], cbar8[:, ds(0, gs), :])
            t4 = trp.tile([P, gs, 64], BF16, name="t4", tag="t4")
            nc.vector.tensor_max(t4[:], s3[:, :, ds(0, 64)], s3[:, :, ds(64, 64)])
            nc.vector.tensor_reduce(
                mx[:, ds(gb, gs)], t4[:], mybir.AxisListType.X, ALU.max
            )
            # epilogue slice for this group while the pipeline continues:
            # dist = sqrt(relu(psq - 2*mx))
            sl = ds(gb, gs)
            nc.vector.scalar_tensor_tensor(
                d2b[:, sl], mx[:, sl], -2.0, psq[:, sl], ALU.mult, ALU.add
            )
            nc.vector.tensor_scalar_max(d2r[:, sl], d2b[:, sl], 0.0)
            nc.scalar.activation(dist[:, sl], d2r[:, sl], AF.Sqrt)

    # --- epilogue: partial = sum_n dist ----------------------------------
    rowsum = const_pool.tile([P, 1], F32, name="rowsum", tag="rowsum")
    nc.vector.tensor_reduce(rowsum[:], dist[:], mybir.AxisListType.X, ALU.add)
    fin = psum_main.tile([1, 1, 1], F32, name="fin", tag="pp", padded_shape=[P, 2, K])
    nc.tensor.matmul(fin[:], rowsum[:], onescol[:], start=True, stop=True)
    out_sb = const_pool.tile([1, 1], F32, name="out_sb", tag="out_sb")
    nc.scalar.copy(out_sb[:], fin[:])
    nc.sync.dma_start(out[:], out_sb[:])


def build():
    nc = bacc.Bacc(
        "TRN2",
        target_bir_lowering=False,
        debug=False,
        enable_asserts=False,
        num_devices=NCORES,
    )
    wall = nc.dram_tensor("wall", [P, MCH, 2, P], FP8, kind="ExternalInput").ap()
    cpack_d = nc.dram_tensor("cpack", [P, 2, K], FP8, kind="ExternalInput").ap()
    cbar8_d = nc.dram_tensor("cbar8", [P, GRP, P], BF16, kind="ExternalInput").ap()
    psqT_d = nc.dram_tensor("psqT", [P, MCH], F32, kind="ExternalInput").ap()
    out = nc.dram_tensor("out", [1, 1], F32, kind="ExternalOutput").ap()
    with tile.TileContext(nc) as tc, ExitStack() as ctx:
        _build_kernel(ctx, tc, out, wall, cpack_d, cbar8_d, psqT_d)
    nc.compile()
    return nc


_NC = None


def _make_in_maps(points: np.ndarray, centers: np.ndarray):
    # column c of cpack holds sorted-rank r = (c % 128)*8 + c//128, so the
    # max-tree block {j, j+128, ..., j+896} covers ranks 8j..8j+7
    cols = np.arange(K)
    rank_of_col = (cols % P) * GRP + cols // P
    in_maps = []
    for b in range(B):
        p8 = points[b].astype(ml_dtypes.float8_e4m3)    # [N, D]
        c8 = centers[b].astype(ml_dtypes.float8_e4m3)   # [K, D]
        pf = p8.astype(np.float32)
        cf = c8.astype(np.float32)
        psq = np.einsum("nd,nd->n", pf, pf)             # [N]
        csqh = 0.5 * np.einsum("kd,kd->k", cf, cf)      # [K]
        order = np.argsort(csqh)
        c8i = c8[order][rank_of_col]                    # centers in column order
        blk = csqh[order].reshape(P, GRP)               # block j = ranks 8j..8j+7
        cbar = 0.5 * (blk.min(1) + blk.max(1))          # [128] midpoint
        cbar8 = np.broadcast_to(
            cbar.astype(ml_dtypes.bfloat16)[None, None, :], (P, GRP, P)
        )
        # wall[p, m, s, n] = p8[m*128+n, s*128+p]
        wall = np.ascontiguousarray(
            p8.reshape(MCH, P, 2, P).transpose(3, 0, 2, 1)
        )
        # cpack[p, s, k] = c8i[k, s*128+p]
        cpack = np.ascontiguousarray(c8i.reshape(K, 2, P).transpose(2, 1, 0))
        in_maps.append(
            {
                "wall": wall,
                "cpack": cpack,
                "cbar8": np.ascontiguousarray(cbar8),
                "psqT": np.ascontiguousarray(psq.reshape(MCH, P).T),
            }
        )
    return in_maps


def kernel(points, centers, **_run_kwargs):
    global _NC
    points = np.asarray(points, dtype=np.float32)
    centers = np.asarray(centers, dtype=np.float32)
    assert points.shape == (B, N, D) and centers.shape == (B, K, D)
    if _NC is None:
        _NC = build()
    res = run_bass_kernel_spmd(
        _NC, _make_in_maps(points, centers), list(range(NCORES)), **_run_kwargs
    )
    total = sum(float(r["out"][0, 0]) for r in res.results)
    return np.array(total / (B * N), dtype=np.float32)


if __name__ == "__main__":
    pts = np.random.RandomState(0).randn(B, N, D).astype(np.float32)
    ctr = np.random.RandomState(1).randn(B, K, D).astype(np.float32)
    print(kernel(pts, ctr))
